# revision 38
# baseline (speedup 1.0000x reference)
"""Trainium2 Bass kernel for quantized BasicBlock (DoReFa conv-bn-act x2 + residual).

Self-contained: builds an 8-core SPMD Bass kernel, shards the batch (64 -> 8x8),
runs via bass_utils.run_bass_kernel_spmd, gathers the full output.

Math (per core, batch shard of 8 images):
  W_int = 2*rint(tanh(w)*s + 7.5) - 15, s = 15/(2*max|tanh(w)|)   (odd ints, |.|<=15)
  conv1: S1 = conv3x3(x, W1_int)            == 15 * conv3x3(x, w_q1)
  BN1 stats of S1 over (N,H,W) all-reduced across cores; eps' = 225e-5
  act1  = clip(rint(S1*sc1 + bi1), 0, 15)   (ints 0..15, stored fp8e4m3)
  conv2: S2 = conv3x3(act1, W2_int)         == 225 * conv3x3(a_q, w_q2), exact int fp32
  BN2 stats of S2 all-reduced; eps'' = 225^2 * 1e-5
  out   = rint(15*clip(S2*sc2 + bi2 + x, 0, 1)) / 15
"""
import sys
from contextlib import ExitStack

import numpy as np

for _p in ("/opt/trn_rl_repo",):
    if _p not in sys.path:
        sys.path.append(_p)

import concourse.bass as bass
import concourse.bass_isa as bass_isa
import concourse.bacc as bacc
import concourse.mybir as mybir
import concourse.tile as tile
from concourse import bass_utils
from concourse.masks import make_identity

F32 = mybir.dt.float32
FP8 = mybir.dt.float8e4

N_CORES = 8
B, C, H, W = 64, 128, 56, 56
BPC = B // N_CORES            # images per core
HP, WP = H + 2, W + 2         # padded 58x58
PW = HP * WP                  # 3364
HW = H * W                    # 3136
RPT = 8                       # output rows per PSUM tile
TN = RPT * W                  # 448 columns per matmul
TPI = H // RPT                # 7 tiles per image
NTILES = BPC * TPI            # 56 tiles per core
N_GLOBAL = float(B * H * W)   # BN population per channel
C23 = float(2 ** 23)
EPS1 = 225.0 * 1e-5           # eps scaled for 15x conv1 output
EPS2 = 225.0 * 225.0 * 1e-5   # eps scaled for 225x conv2 output

MM1_DT = mybir.dt.float32     # conv1 matmul dtype (float32 | float32r)
DEBUG = False                 # adds intermediate-dump outputs

TAPS = [(dy, dx) for dy in range(3) for dx in range(3)]

_CACHE = {}


def _quant_weights(nc, ctx, tc, pools, w_in, name):
    """DMA + DoReFa-quantize weights; returns fp32 W_int in natural (O, I*9) layout.

    All elementwise steps are in-place on one (C, C*9) tile.
    """
    wp = pools["wprep"]
    wk = wp.tile([C, C * 9], F32, name=f"{name}_wk", tag="wk")
    nc.sync.dma_start(wk[:], w_in[:])
    nc.scalar.activation(wk[:], wk[:], mybir.ActivationFunctionType.Tanh)
    # global max|tanh(w)| broadcast to all partitions
    am = wp.tile([C, 1], F32, name=f"{name}_am", tag="wam")
    nc.vector.tensor_reduce(am[:], wk[:], axis=mybir.AxisListType.X,
                            op=mybir.AluOpType.max, apply_absolute_value=True)
    amg = wp.tile([C, 1], F32, name=f"{name}_amg", tag="wamg")
    nc.gpsimd.partition_all_reduce(amg[:], am[:], channels=C,
                                   reduce_op=bass_isa.ReduceOp.max)
    s_t = wp.tile([C, 1], F32, name=f"{name}_s", tag="ws")
    nc.vector.reciprocal(s_t[:], amg[:])
    nc.vector.tensor_scalar_mul(s_t[:], s_t[:], 7.5)
    # W_int = 2*rint(tanh*s + 7.5) - 15  (rint via +2^23-2^23; 7.5 added
    # separately — 2^23+7.5 is not representable in fp32)
    nc.vector.tensor_scalar(wk[:], wk[:], s_t[:], 7.5,
                            op0=mybir.AluOpType.mult, op1=mybir.AluOpType.add)
    nc.vector.tensor_scalar(wk[:], wk[:], C23, C23,
                            op0=mybir.AluOpType.add, op1=mybir.AluOpType.subtract)
    nc.vector.tensor_scalar(wk[:], wk[:], 2.0, 15.0,
                            op0=mybir.AluOpType.mult, op1=mybir.AluOpType.subtract)
    return wk


def _transpose_taps(nc, pools, wint, identity, out_dt, name):
    """Per-tap PE transpose of W_int (O,(I,t)) -> wT (I,(t,O)) in out_dt."""
    wp = pools["wconst"]
    trp = pools["psum_tr"]
    wT = wp.tile([C, 9 * C], out_dt, name=f"{name}_T")
    wr = wint.rearrange("p (i t) -> p i t", t=9)
    for t in range(9):
        ps = trp.tile([C, C], F32, name=f"{name}_ps{t}", tag="trps")
        nc.tensor.transpose(ps[:], wr[:, :, t], identity[:])
        nc.scalar.copy(wT[:, t * C:(t + 1) * C], ps[:])
    return wT


def _conv_image(nc, pools, src_pad, wT, stats, out_sb, img_idx, name):
    """One image: 7 PSUM tiles x 9 accumulating taps; bn_stats + copy per tile."""
    cp = pools["psum_conv"]
    srcr = src_pad.rearrange("p (h w) -> p h w", w=WP)
    for t in range(TPI):
        gi = img_idx * TPI + t
        ps = cp.tile([C, TN], F32, name=f"{name}_ps", tag="convps")
        for k, (dy, dx) in enumerate(TAPS):
            lhsT = wT[:, k * C:(k + 1) * C]
            rhs = srcr[:, RPT * t + dy: RPT * t + dy + RPT, dx: dx + W]
            nc.tensor.matmul(ps[:], lhsT, rhs, start=(k == 0), stop=(k == 8))
        nc.vector.bn_stats(stats[:, gi * 6:(gi + 1) * 6], ps[:])
        nc.scalar.copy(out_sb[:, t * TN:(t + 1) * TN], ps[:])


def _bn_allreduce(nc, pools, stats, k_scale, cc_idx, dbg=None):
    """bn_aggr local stats (of k*S) -> unscaled (mean, E[x^2])/8 payload ->
    AllReduce -> mean_u, rstd_u in UNSCALED units (good Sqrt-LUT range)."""
    eps = 1e-5
    sp = pools["stats"]
    dp = pools["dram"]
    loc = sp.tile([C, 2], F32, name=f"bn{cc_idx}_loc")
    nc.vector.bn_aggr(loc[:], stats.rearrange("p (t k) -> p t k", k=3))
    # rescale to unscaled units: mean /= k, var /= k^2
    nc.vector.tensor_scalar(loc[:, 0:1], loc[:, 0:1], 1.0 / k_scale, None,
                            op0=mybir.AluOpType.mult)
    nc.vector.tensor_scalar(loc[:, 1:2], loc[:, 1:2], 1.0 / (k_scale * k_scale),
                            None, op0=mybir.AluOpType.mult)
    if dbg is not None:
        nc.sync.dma_start(dbg[:, 0:2], loc[:])
    pay = sp.tile([C, 2], F32, name=f"bn{cc_idx}_pay")
    # pay0 = mean/8 ; pay1 = (var + mean^2)/8
    nc.vector.tensor_scalar_mul(pay[:, 0:1], loc[:, 0:1], 1.0 / N_CORES)
    msq = sp.tile([C, 1], F32, name=f"bn{cc_idx}_msq")
    nc.vector.scalar_tensor_tensor(msq[:], loc[:, 0:1], 1.0 / N_CORES, loc[:, 0:1],
                                   op0=mybir.AluOpType.mult, op1=mybir.AluOpType.mult)
    nc.vector.scalar_tensor_tensor(pay[:, 1:2], loc[:, 1:2], 1.0 / N_CORES, msq[:],
                                   op0=mybir.AluOpType.mult, op1=mybir.AluOpType.add)
    cc_in = dp.tile([C, 2], F32, name=f"cc{cc_idx}_in")
    cc_out = dp.tile([C, 2], F32, name=f"cc{cc_idx}_out")
    nc.sync.dma_start(cc_in[:], pay[:])
    nc.gpsimd.collective_compute(
        "AllReduce", mybir.AluOpType.add,
        replica_groups=[list(range(N_CORES))],
        ins=[cc_in.opt()], outs=[cc_out.opt()],
    )
    gs = sp.tile([C, 2], F32, name=f"bn{cc_idx}_gs")
    nc.sync.dma_start(gs[:], cc_out[:])
    if dbg is not None:
        nc.sync.dma_start(dbg[:, 2:4], gs[:])
    mean_g = gs[:, 0:1]
    # var_g = E[x^2] - mean^2 ; rstd = 1/sqrt(var_g + eps)
    m2 = sp.tile([C, 1], F32, name=f"bn{cc_idx}_m2")
    nc.vector.scalar_tensor_tensor(m2[:], mean_g, 1.0, mean_g,
                                   op0=mybir.AluOpType.mult, op1=mybir.AluOpType.mult)
    varg = sp.tile([C, 1], F32, name=f"bn{cc_idx}_var")
    nc.vector.scalar_tensor_tensor(varg[:], m2[:], -1.0, gs[:, 1:2],
                                   op0=mybir.AluOpType.mult, op1=mybir.AluOpType.add)
    epst = sp.tile([C, 1], F32, name=f"bn{cc_idx}_eps")
    nc.vector.memset(epst[:], eps)
    std = sp.tile([C, 1], F32, name=f"bn{cc_idx}_std")
    nc.scalar.activation(std[:], varg[:], mybir.ActivationFunctionType.Sqrt,
                         bias=epst[:])
    rstd = sp.tile([C, 1], F32, name=f"bn{cc_idx}_rstd")
    nc.vector.reciprocal(rstd[:], std[:])
    return mean_g, rstd


def _affine_vecs(nc, pools, gamma, beta, mean_u, rstd_u, m_out, k_scale, cc_idx):
    """For y_out = m*bn(S/k): sc = m*gamma*rstd/k ; bi = m*(beta - mean_u*gamma*rstd).

    mean_u / rstd_u are in unscaled units; S is the k-scaled PSUM value.
    """
    sp = pools["stats"]
    gr = sp.tile([C, 1], F32, name=f"gr{cc_idx}")
    nc.vector.scalar_tensor_tensor(gr[:], gamma[:], 1.0, rstd_u[:],
                                   op0=mybir.AluOpType.bypass,
                                   op1=mybir.AluOpType.mult)
    sc = sp.tile([C, 1], F32, name=f"sc{cc_idx}")
    nc.vector.tensor_scalar_mul(sc[:], gr[:], m_out / k_scale)
    negms = sp.tile([C, 1], F32, name=f"negms{cc_idx}")
    nc.vector.scalar_tensor_tensor(negms[:], mean_u, -1.0, gr[:],
                                   op0=mybir.AluOpType.mult, op1=mybir.AluOpType.mult)
    bi = sp.tile([C, 1], F32, name=f"bi{cc_idx}")
    nc.vector.scalar_tensor_tensor(bi[:], negms[:], 1.0, beta[:],
                                   op0=mybir.AluOpType.bypass,
                                   op1=mybir.AluOpType.add)
    # bi currently = (-mean*gr) + beta ; scale by m
    nc.vector.tensor_scalar_mul(bi[:], bi[:], m_out)
    return sc, bi


def build():
    nc = bacc.Bacc("TRN2", target_bir_lowering=False, debug=False,
                   enable_asserts=False, num_devices=N_CORES)
    x_in = nc.dram_tensor("x", [BPC, C, H, W], F32, kind="ExternalInput").ap()
    w1_in = nc.dram_tensor("w1", [C, C * 9], F32, kind="ExternalInput").ap()
    w2_in = nc.dram_tensor("w2", [C, C * 9], F32, kind="ExternalInput").ap()
    g1_in = nc.dram_tensor("gamma1", [C, 1], F32, kind="ExternalInput").ap()
    b1_in = nc.dram_tensor("beta1", [C, 1], F32, kind="ExternalInput").ap()
    g2_in = nc.dram_tensor("gamma2", [C, 1], F32, kind="ExternalInput").ap()
    b2_in = nc.dram_tensor("beta2", [C, 1], F32, kind="ExternalInput").ap()
    out_d = nc.dram_tensor("out", [BPC, C, H, W], F32, kind="ExternalOutput").ap()
    if DEBUG:
        dbg_w1 = nc.dram_tensor("dbg_w1", [C, C * 9], F32, kind="ExternalOutput").ap()
        dbg_o1 = nc.dram_tensor("dbg_o1", [C, HW], F32, kind="ExternalOutput").ap()
        dbg_st = nc.dram_tensor("dbg_st", [C, 8], F32, kind="ExternalOutput").ap()
        dbg_a1 = nc.dram_tensor("dbg_a1", [C, PW], F32, kind="ExternalOutput").ap()
        dbg_o2 = nc.dram_tensor("dbg_o2", [C, HW], F32, kind="ExternalOutput").ap()
        dbg_ab = nc.dram_tensor("dbg_ab", [C, 4], F32, kind="ExternalOutput").ap()

    with tile.TileContext(nc) as tc, ExitStack() as ctx:
        pools = {
            "wprep": ctx.enter_context(tc.tile_pool(name="wprep", bufs=1)),
            "wconst": ctx.enter_context(tc.tile_pool(name="wconst", bufs=1)),
            "stats": ctx.enter_context(tc.tile_pool(name="stats", bufs=1)),
            "big": ctx.enter_context(tc.tile_pool(name="big", bufs=8)),
            "xpad": ctx.enter_context(tc.tile_pool(name="xpad", bufs=2)),
            "a1pad": ctx.enter_context(tc.tile_pool(name="a1pad", bufs=1)),
            "tail": ctx.enter_context(tc.tile_pool(name="tail", bufs=2)),
            "psum_conv": ctx.enter_context(
                tc.tile_pool(name="psum_conv", bufs=7, space="PSUM")),
            "psum_tr": ctx.enter_context(
                tc.tile_pool(name="psum_tr", bufs=1, space="PSUM")),
            "dram": ctx.enter_context(tc.tile_pool(name="dram", bufs=4, space="DRAM")),
        }
        consts = pools["wconst"]

        # per-channel params
        g1 = consts.tile([C, 1], F32, name="g1"); nc.sync.dma_start(g1[:], g1_in[:])
        b1 = consts.tile([C, 1], F32, name="b1"); nc.sync.dma_start(b1[:], b1_in[:])
        g2 = consts.tile([C, 1], F32, name="g2"); nc.sync.dma_start(g2[:], g2_in[:])
        b2 = consts.tile([C, 1], F32, name="b2"); nc.sync.dma_start(b2[:], b2_in[:])

        identity = consts.tile([C, C], F32, name="identity")
        make_identity(nc, identity[:])

        # ---- weights ----
        w1i = _quant_weights(nc, ctx, tc, pools, w1_in, "w1")
        if DEBUG:
            nc.sync.dma_start(dbg_w1[:], w1i[:])
        w1T = _transpose_taps(nc, pools, w1i, identity, MM1_DT, "w1")
        w2i = _quant_weights(nc, ctx, tc, pools, w2_in, "w2")
        w2T = _transpose_taps(nc, pools, w2i, identity, FP8, "w2")

        # ---- phase A: conv1 per image ----
        stats1 = pools["stats"].tile([C, NTILES * 6], F32, name="stats1")
        out1 = []
        for n in range(BPC):
            xp = pools["xpad"].tile([C, PW], MM1_DT, name="xp", tag="xp")
            xpr = xp.rearrange("p (h w) -> p h w", w=WP)
            if MM1_DT == F32:
                nc.gpsimd.memset(xp[:], 0.0)
                nc.sync.dma_start(xpr[:, 1:1 + H, 1:1 + W], x_in[n])
            else:
                # fp32r inputs must be produced by a rounding compute engine
                nc.vector.memset(xpr[:, 0, :], 0.0)
                nc.vector.memset(xpr[:, HP - 1, :], 0.0)
                nc.vector.memset(xpr[:, 1:1 + H, 0], 0.0)
                nc.vector.memset(xpr[:, 1:1 + H, WP - 1], 0.0)
                xs = pools["tail"].tile([C, HW], F32, name="xs", tag="tailbuf")
                nc.sync.dma_start(xs[:], x_in[n])
                nc.scalar.copy(xpr[:, 1:1 + H, 1:1 + W],
                               xs.rearrange("p (h w) -> p h w", w=W))
            o1 = pools["big"].tile([C, HW], F32, name=f"o1_{n}", tag="bigbuf")
            _conv_image(nc, pools, xp, w1T, stats1, o1, n, "c1")
            out1.append(o1)

        if DEBUG:
            nc.sync.dma_start(dbg_o1[:], out1[0][:])

        # ---- BN1 all-reduce + act1 ----
        mean1, rstd1 = _bn_allreduce(nc, pools, stats1, 15.0, 1,
                                     dbg=dbg_st[:, 0:4] if DEBUG else None)
        sc1, bi1 = _affine_vecs(nc, pools, g1, b1, mean1, rstd1, 15.0, 15.0, 1)

        a1_8 = pools["a1pad"].tile([C, BPC * PW], FP8, name="a1pad")
        nc.gpsimd.memset(a1_8[:], 0.0)

        # ---- phase B: act1 + conv2 per image ----
        stats2 = pools["stats"].tile([C, NTILES * 6], F32, name="stats2")
        out2 = []
        for n in range(BPC):
            o1 = out1[n]
            aff = pools["tail"].tile([C, HW], F32, name="aff", tag="tailbuf")
            nc.vector.tensor_scalar(aff[:], o1[:], sc1[:], bi1[:],
                                    op0=mybir.AluOpType.mult,
                                    op1=mybir.AluOpType.add)
            # clamp + rint -> fp8 interior of padded act1
            clp = pools["tail"].tile([C, HW], F32, name="clp", tag="tailbuf")
            nc.vector.tensor_scalar(clp[:], aff[:], 0.0, 15.0,
                                    op0=mybir.AluOpType.max, op1=mybir.AluOpType.min)
            ap8 = a1_8.rearrange("p (n h w) -> p n h w", n=BPC, w=WP)
            nc.vector.tensor_scalar(
                ap8[:, n, 1:1 + H, 1:1 + W],
                clp.rearrange("p (h w) -> p h w", w=W), C23, C23,
                op0=mybir.AluOpType.add, op1=mybir.AluOpType.subtract)
            o2 = pools["big"].tile([C, HW], F32, name=f"o2_{n}", tag="bigbuf")
            apn = a1_8[:, n * PW:(n + 1) * PW]
            _conv_image(nc, pools, apn, w2T, stats2, o2, n, "c2")
            out2.append(o2)

        if DEBUG:
            a1f = pools["xpad"].tile([C, PW], F32, name="a1f", tag="xp")
            nc.scalar.copy(a1f[:], a1_8[:, 0:PW])
            nc.sync.dma_start(dbg_a1[:], a1f[:])
            nc.sync.dma_start(dbg_o2[:], out2[0][:])

        # ---- BN2 all-reduce + tail ----
        mean2, rstd2 = _bn_allreduce(nc, pools, stats2, 225.0, 2,
                                     dbg=dbg_st[:, 4:8] if DEBUG else None)
        sc2, bi2 = _affine_vecs(nc, pools, g2, b2, mean2, rstd2, 1.0, 225.0, 2)

        if DEBUG:
            nc.sync.dma_start(dbg_ab[:, 0:1], sc1[:])
            nc.sync.dma_start(dbg_ab[:, 1:2], bi1[:])
            nc.sync.dma_start(dbg_ab[:, 2:3], sc2[:])
            nc.sync.dma_start(dbg_ab[:, 3:4], bi2[:])

        for n in range(BPC):
            o2 = out2[n]
            xr = pools["xpad"].tile([C, HW], F32, name="xr", tag="xp")
            nc.sync.dma_start(xr[:], x_in[n])
            u = pools["tail"].tile([C, HW], F32, name="u", tag="tailbuf")
            nc.vector.tensor_scalar(u[:], o2[:], sc2[:], bi2[:],
                                    op0=mybir.AluOpType.mult,
                                    op1=mybir.AluOpType.add)
            nc.vector.scalar_tensor_tensor(u[:], u[:], 1.0, xr[:],
                                           op0=mybir.AluOpType.bypass,
                                           op1=mybir.AluOpType.add)
            nc.vector.tensor_scalar(u[:], u[:], 0.0, 1.0,
                                    op0=mybir.AluOpType.max, op1=mybir.AluOpType.min)
            nc.vector.tensor_scalar(u[:], u[:], 15.0, C23,
                                    op0=mybir.AluOpType.mult, op1=mybir.AluOpType.add)
            nc.vector.tensor_scalar(u[:], u[:], C23, 1.0 / 15.0,
                                    op0=mybir.AluOpType.subtract,
                                    op1=mybir.AluOpType.mult)
            nc.sync.dma_start(out_d[n], u.rearrange("p (h w) -> p h w", w=W))

    nc.compile()
    return nc


def _get_nc():
    if "nc" not in _CACHE:
        _CACHE["nc"] = build()
    return _CACHE["nc"]


def kernel(x, w1, w2, gamma1, beta1, gamma2, beta2, _trace=False):
    nc = _get_nc()
    x = np.ascontiguousarray(np.asarray(x, dtype=np.float32))
    in_common = {
        "w1": np.ascontiguousarray(np.asarray(w1, np.float32).reshape(C, C * 9)),
        "w2": np.ascontiguousarray(np.asarray(w2, np.float32).reshape(C, C * 9)),
        "gamma1": np.asarray(gamma1, np.float32).reshape(C, 1),
        "beta1": np.asarray(beta1, np.float32).reshape(C, 1),
        "gamma2": np.asarray(gamma2, np.float32).reshape(C, 1),
        "beta2": np.asarray(beta2, np.float32).reshape(C, 1),
    }
    in_maps = [dict(in_common, x=x[c * BPC:(c + 1) * BPC]) for c in range(N_CORES)]
    res = bass_utils.run_bass_kernel_spmd(nc, in_maps, core_ids=list(range(N_CORES)),
                                          trace=_trace)
    out = np.concatenate([res.results[c]["out"] for c in range(N_CORES)], axis=0)
    if _trace:
        _CACHE["last_exec_time_ns"] = res.exec_time_ns
        _CACHE["last_results"] = res
    return out


if __name__ == "__main__":
    nc = build()
    print("built ok")


# revision 50
# speedup vs baseline: 1.1100x; 1.1100x over previous
"""Trainium2 Bass kernel for quantized BasicBlock (DoReFa conv-bn-act x2 + residual).

Self-contained: builds an 8-core SPMD Bass kernel, shards the batch (64 -> 8x8),
runs via bass_utils.run_bass_kernel_spmd, gathers the full output.

Math (per core, batch shard of 8 images):
  W_int = 2*rint(tanh(w)*s + 7.5) - 15, s = 15/(2*max|tanh(w)|)   (odd ints, |.|<=15)
  conv1: S1 = conv3x3(x, W1_int)            == 15 * conv3x3(x, w_q1)
  BN1 stats of S1 over (N,H,W) all-reduced across cores; eps' = 225e-5
  act1  = clip(rint(S1*sc1 + bi1), 0, 15)   (ints 0..15, stored fp8e4m3)
  conv2: S2 = conv3x3(act1, W2_int)         == 225 * conv3x3(a_q, w_q2), exact int fp32
  BN2 stats of S2 all-reduced; eps'' = 225^2 * 1e-5
  out   = rint(15*clip(S2*sc2 + bi2 + x, 0, 1)) / 15
"""
import sys
from contextlib import ExitStack

import numpy as np

for _p in ("/opt/trn_rl_repo",):
    if _p not in sys.path:
        sys.path.append(_p)

import concourse.bass as bass
import concourse.bass_isa as bass_isa
import concourse.bacc as bacc
import concourse.mybir as mybir
import concourse.tile as tile
from concourse import bass_utils
from concourse.masks import make_identity

F32 = mybir.dt.float32
FP8 = mybir.dt.float8e4

N_CORES = 8
B, C, H, W = 64, 128, 56, 56
BPC = B // N_CORES            # images per core
HP, WP = H + 2, W + 2         # padded 58x58
PW = HP * WP                  # 3364
HW = H * W                    # 3136
RPT = 8                       # output rows per PSUM tile
TN = RPT * W                  # 448 columns per matmul
TPI = H // RPT                # 7 tiles per image
NTILES = BPC * TPI            # 56 tiles per core
N_GLOBAL = float(B * H * W)   # BN population per channel
C23 = float(2 ** 23)
EPS1 = 225.0 * 1e-5           # eps scaled for 15x conv1 output
EPS2 = 225.0 * 225.0 * 1e-5   # eps scaled for 225x conv2 output

MM1_DT = mybir.dt.float32r    # conv1 matmul dtype (float32 | float32r)
DEBUG = False                 # adds intermediate-dump outputs

TAPS = [(dy, dx) for dy in range(3) for dx in range(3)]

_CACHE = {}


def _quant_weights(nc, ctx, tc, pools, w_in, name):
    """DMA + DoReFa-quantize weights; returns fp32 W_int in natural (O, I*9) layout.

    All elementwise steps are in-place on one (C, C*9) tile.
    """
    wp = pools["wprep"]
    wk = wp.tile([C, C * 9], F32, name=f"{name}_wk", tag="wk")
    nc.sync.dma_start(wk[:], w_in[:])
    nc.scalar.activation(wk[:], wk[:], mybir.ActivationFunctionType.Tanh)
    # global max|tanh(w)| broadcast to all partitions
    am = wp.tile([C, 1], F32, name=f"{name}_am", tag="wam")
    nc.vector.tensor_reduce(am[:], wk[:], axis=mybir.AxisListType.X,
                            op=mybir.AluOpType.max, apply_absolute_value=True)
    amg = wp.tile([C, 1], F32, name=f"{name}_amg", tag="wamg")
    nc.gpsimd.partition_all_reduce(amg[:], am[:], channels=C,
                                   reduce_op=bass_isa.ReduceOp.max)
    s_t = wp.tile([C, 1], F32, name=f"{name}_s", tag="ws")
    nc.vector.reciprocal(s_t[:], amg[:])
    nc.vector.tensor_scalar_mul(s_t[:], s_t[:], 7.5)
    # W_int = 2*rint(tanh*s + 7.5) - 15  (rint via +2^23-2^23; 7.5 added
    # separately — 2^23+7.5 is not representable in fp32)
    nc.vector.tensor_scalar(wk[:], wk[:], s_t[:], 7.5,
                            op0=mybir.AluOpType.mult, op1=mybir.AluOpType.add)
    nc.vector.tensor_scalar(wk[:], wk[:], C23, C23,
                            op0=mybir.AluOpType.add, op1=mybir.AluOpType.subtract)
    nc.vector.tensor_scalar(wk[:], wk[:], 2.0, 15.0,
                            op0=mybir.AluOpType.mult, op1=mybir.AluOpType.subtract)
    return wk


def _transpose_taps(nc, pools, wint, identity, out_dt, name):
    """Per-tap PE transpose of W_int (O,(I,t)) -> wT (I,(t,O)) in out_dt."""
    wp = pools["wconst"]
    trp = pools["psum_tr"]
    wT = wp.tile([C, 9 * C], out_dt, name=f"{name}_T")
    wr = wint.rearrange("p (i t) -> p i t", t=9)
    for t in range(9):
        ps = trp.tile([C, C], F32, name=f"{name}_ps{t}", tag="trps")
        nc.tensor.transpose(ps[:], wr[:, :, t], identity[:])
        nc.scalar.copy(wT[:, t * C:(t + 1) * C], ps[:])
    return wT


def _conv_image(nc, pools, src_pad, wT, stats, out_sb, img_idx, name):
    """One image: 7 PSUM tiles x 9 accumulating taps; bn_stats + copy per tile."""
    cp = pools["psum_conv"]
    srcr = src_pad.rearrange("p (h w) -> p h w", w=WP)
    for t in range(TPI):
        gi = img_idx * TPI + t
        ps = cp.tile([C, TN], F32, name=f"{name}_ps", tag="convps")
        for k, (dy, dx) in enumerate(TAPS):
            lhsT = wT[:, k * C:(k + 1) * C]
            rhs = srcr[:, RPT * t + dy: RPT * t + dy + RPT, dx: dx + W]
            nc.tensor.matmul(ps[:], lhsT, rhs, start=(k == 0), stop=(k == 8))
        nc.vector.bn_stats(stats[:, gi * 6:(gi + 1) * 6], ps[:])
        nc.scalar.copy(out_sb[:, t * TN:(t + 1) * TN], ps[:])


def _warmup_allreduce_eps(nc, pools):
    """Tiny AllReduce at kernel start: warms up ncfw (first collective pays a
    large one-time latency) and produces the BN epsilon constant (8 * 1e-5/8),
    so it has a live consumer and survives DCE."""
    sp = pools["stats"]
    dp = pools["dram"]
    eps8 = sp.tile([C, 1], F32, name="eps8")
    nc.vector.memset(eps8[:], 1e-5 / N_CORES)
    cc_in = dp.tile([C, 1], F32, name="ccw_in")
    cc_out = dp.tile([C, 1], F32, name="ccw_out")
    nc.sync.dma_start(cc_in[:], eps8[:])
    nc.gpsimd.collective_compute(
        "AllReduce", mybir.AluOpType.add,
        replica_groups=[list(range(N_CORES))],
        ins=[cc_in.opt()], outs=[cc_out.opt()],
    )
    epst = sp.tile([C, 1], F32, name="epst")
    nc.sync.dma_start(epst[:], cc_out[:])
    return epst


def _bn_allreduce(nc, pools, stats, k_scale, cc_idx, epst, dbg=None):
    """bn_aggr local stats (of k*S) -> unscaled (mean, E[x^2])/8 payload ->
    AllReduce -> mean_u, rstd_u in UNSCALED units (good Sqrt-LUT range)."""
    sp = pools["stats"]
    dp = pools["dram"]
    loc = sp.tile([C, 2], F32, name=f"bn{cc_idx}_loc")
    nc.vector.bn_aggr(loc[:], stats.rearrange("p (t k) -> p t k", k=3))
    # rescale to unscaled units: mean /= k, var /= k^2
    nc.vector.tensor_scalar(loc[:, 0:1], loc[:, 0:1], 1.0 / k_scale, None,
                            op0=mybir.AluOpType.mult)
    nc.vector.tensor_scalar(loc[:, 1:2], loc[:, 1:2], 1.0 / (k_scale * k_scale),
                            None, op0=mybir.AluOpType.mult)
    if dbg is not None:
        nc.sync.dma_start(dbg[:, 0:2], loc[:])
    pay = sp.tile([C, 2], F32, name=f"bn{cc_idx}_pay")
    # pay0 = mean/8 ; pay1 = (var + mean^2)/8
    nc.vector.tensor_scalar_mul(pay[:, 0:1], loc[:, 0:1], 1.0 / N_CORES)
    msq = sp.tile([C, 1], F32, name=f"bn{cc_idx}_msq")
    nc.vector.scalar_tensor_tensor(msq[:], loc[:, 0:1], 1.0 / N_CORES, loc[:, 0:1],
                                   op0=mybir.AluOpType.mult, op1=mybir.AluOpType.mult)
    nc.vector.scalar_tensor_tensor(pay[:, 1:2], loc[:, 1:2], 1.0 / N_CORES, msq[:],
                                   op0=mybir.AluOpType.mult, op1=mybir.AluOpType.add)
    cc_in = dp.tile([C, 2], F32, name=f"cc{cc_idx}_in")
    cc_out = dp.tile([C, 2], F32, name=f"cc{cc_idx}_out")
    nc.sync.dma_start(cc_in[:], pay[:])
    nc.gpsimd.collective_compute(
        "AllReduce", mybir.AluOpType.add,
        replica_groups=[list(range(N_CORES))],
        ins=[cc_in.opt()], outs=[cc_out.opt()],
    )
    gs = sp.tile([C, 2], F32, name=f"bn{cc_idx}_gs")
    nc.sync.dma_start(gs[:], cc_out[:])
    if dbg is not None:
        nc.sync.dma_start(dbg[:, 2:4], gs[:])
    mean_g = gs[:, 0:1]
    # var_g = E[x^2] - mean^2 ; rstd = 1/sqrt(var_g + eps)
    m2 = sp.tile([C, 1], F32, name=f"bn{cc_idx}_m2")
    nc.vector.scalar_tensor_tensor(m2[:], mean_g, 1.0, mean_g,
                                   op0=mybir.AluOpType.mult, op1=mybir.AluOpType.mult)
    varg = sp.tile([C, 1], F32, name=f"bn{cc_idx}_var")
    nc.vector.scalar_tensor_tensor(varg[:], m2[:], -1.0, gs[:, 1:2],
                                   op0=mybir.AluOpType.mult, op1=mybir.AluOpType.add)
    std = sp.tile([C, 1], F32, name=f"bn{cc_idx}_std")
    nc.scalar.activation(std[:], varg[:], mybir.ActivationFunctionType.Sqrt,
                         bias=epst[:])
    rstd = sp.tile([C, 1], F32, name=f"bn{cc_idx}_rstd")
    nc.vector.reciprocal(rstd[:], std[:])
    return mean_g, rstd


def _affine_vecs(nc, pools, gamma, beta, mean_u, rstd_u, m_out, k_scale, cc_idx):
    """For y_out = m*bn(S/k): sc = m*gamma*rstd/k ; bi = m*(beta - mean_u*gamma*rstd).

    mean_u / rstd_u are in unscaled units; S is the k-scaled PSUM value.
    """
    sp = pools["stats"]
    gr = sp.tile([C, 1], F32, name=f"gr{cc_idx}")
    nc.vector.scalar_tensor_tensor(gr[:], gamma[:], 1.0, rstd_u[:],
                                   op0=mybir.AluOpType.bypass,
                                   op1=mybir.AluOpType.mult)
    sc = sp.tile([C, 1], F32, name=f"sc{cc_idx}")
    nc.vector.tensor_scalar_mul(sc[:], gr[:], m_out / k_scale)
    negms = sp.tile([C, 1], F32, name=f"negms{cc_idx}")
    nc.vector.scalar_tensor_tensor(negms[:], mean_u, -1.0, gr[:],
                                   op0=mybir.AluOpType.mult, op1=mybir.AluOpType.mult)
    bi = sp.tile([C, 1], F32, name=f"bi{cc_idx}")
    nc.vector.scalar_tensor_tensor(bi[:], negms[:], 1.0, beta[:],
                                   op0=mybir.AluOpType.bypass,
                                   op1=mybir.AluOpType.add)
    # bi currently = (-mean*gr) + beta ; scale by m
    nc.vector.tensor_scalar_mul(bi[:], bi[:], m_out)
    return sc, bi


def build():
    nc = bacc.Bacc("TRN2", target_bir_lowering=False, debug=False,
                   enable_asserts=False, num_devices=N_CORES)
    x_in = nc.dram_tensor("x", [BPC, C, H, W], F32, kind="ExternalInput").ap()
    w1_in = nc.dram_tensor("w1", [C, C * 9], F32, kind="ExternalInput").ap()
    w2_in = nc.dram_tensor("w2", [C, C * 9], F32, kind="ExternalInput").ap()
    g1_in = nc.dram_tensor("gamma1", [C, 1], F32, kind="ExternalInput").ap()
    b1_in = nc.dram_tensor("beta1", [C, 1], F32, kind="ExternalInput").ap()
    g2_in = nc.dram_tensor("gamma2", [C, 1], F32, kind="ExternalInput").ap()
    b2_in = nc.dram_tensor("beta2", [C, 1], F32, kind="ExternalInput").ap()
    out_d = nc.dram_tensor("out", [BPC, C, H, W], F32, kind="ExternalOutput").ap()
    if DEBUG:
        dbg_w1 = nc.dram_tensor("dbg_w1", [C, C * 9], F32, kind="ExternalOutput").ap()
        dbg_o1 = nc.dram_tensor("dbg_o1", [C, HW], F32, kind="ExternalOutput").ap()
        dbg_st = nc.dram_tensor("dbg_st", [C, 8], F32, kind="ExternalOutput").ap()
        dbg_a1 = nc.dram_tensor("dbg_a1", [C, PW], F32, kind="ExternalOutput").ap()
        dbg_o2 = nc.dram_tensor("dbg_o2", [C, HW], F32, kind="ExternalOutput").ap()
        dbg_ab = nc.dram_tensor("dbg_ab", [C, 4], F32, kind="ExternalOutput").ap()

    with tile.TileContext(nc) as tc, ExitStack() as ctx:
        pools = {
            "wprep": ctx.enter_context(tc.tile_pool(name="wprep", bufs=1)),
            "wconst": ctx.enter_context(tc.tile_pool(name="wconst", bufs=1)),
            "stats": ctx.enter_context(tc.tile_pool(name="stats", bufs=1)),
            "big": ctx.enter_context(tc.tile_pool(name="big", bufs=8)),
            "xpad": ctx.enter_context(tc.tile_pool(name="xpad", bufs=2)),
            "a1pad": ctx.enter_context(tc.tile_pool(name="a1pad", bufs=1)),
            "tail": ctx.enter_context(tc.tile_pool(name="tail", bufs=2)),
            "psum_conv": ctx.enter_context(
                tc.tile_pool(name="psum_conv", bufs=7, space="PSUM")),
            "psum_tr": ctx.enter_context(
                tc.tile_pool(name="psum_tr", bufs=1, space="PSUM")),
            "dram": ctx.enter_context(tc.tile_pool(name="dram", bufs=4, space="DRAM")),
        }
        consts = pools["wconst"]

        # per-channel params
        g1 = consts.tile([C, 1], F32, name="g1"); nc.sync.dma_start(g1[:], g1_in[:])
        b1 = consts.tile([C, 1], F32, name="b1"); nc.sync.dma_start(b1[:], b1_in[:])
        g2 = consts.tile([C, 1], F32, name="g2"); nc.sync.dma_start(g2[:], g2_in[:])
        b2 = consts.tile([C, 1], F32, name="b2"); nc.sync.dma_start(b2[:], b2_in[:])

        identity = consts.tile([C, C], F32, name="identity")
        make_identity(nc, identity[:])

        epst = _warmup_allreduce_eps(nc, pools)

        zsrc = consts.tile([C, 2 * WP], F32, name="zsrc")
        nc.vector.memset(zsrc[:], 0.0)

        # ---- weights ----
        w1i = _quant_weights(nc, ctx, tc, pools, w1_in, "w1")
        if DEBUG:
            nc.sync.dma_start(dbg_w1[:], w1i[:])
        w1T = _transpose_taps(nc, pools, w1i, identity, MM1_DT, "w1")
        w2i = _quant_weights(nc, ctx, tc, pools, w2_in, "w2")
        w2T = _transpose_taps(nc, pools, w2i, identity, FP8, "w2")

        # ---- phase A: conv1 per image ----
        stats1 = pools["stats"].tile([C, NTILES * 6], F32, name="stats1")
        out1 = []
        for n in range(BPC):
            xp = pools["xpad"].tile([C, PW], MM1_DT, name="xp", tag="xp")
            xpr = xp.rearrange("p (h w) -> p h w", w=WP)
            # halo-only zeroing: top row, bottom row, and (right col, next
            # row's left col) pairs which are memory-adjacent.  ACT copies
            # from a zero tile so the fp32r producer rule holds.
            nc.scalar.copy(xpr[:, 0, :], zsrc[:, 0:WP])
            nc.scalar.copy(xpr[:, HP - 1, :], zsrc[:, 0:WP])
            side = xp[:, WP - 1:WP - 1 + (HP - 1) * WP].rearrange(
                "p (a b) -> p a b", b=WP)
            nc.scalar.copy(side[:, :, 0:2],
                           zsrc[:, 0:2 * (HP - 1)].rearrange(
                               "p (a b) -> p a b", b=2))
            if MM1_DT == F32:
                nc.sync.dma_start(xpr[:, 1:1 + H, 1:1 + W], x_in[n])
            else:
                # fp32r inputs must be produced by a rounding compute engine
                xs = pools["tail"].tile([C, HW], F32, name="xs", tag="tailbuf")
                nc.sync.dma_start(xs[:], x_in[n])
                nc.scalar.copy(xpr[:, 1:1 + H, 1:1 + W],
                               xs.rearrange("p (h w) -> p h w", w=W))
            o1 = pools["big"].tile([C, HW], F32, name=f"o1_{n}", tag="bigbuf")
            _conv_image(nc, pools, xp, w1T, stats1, o1, n, "c1")
            out1.append(o1)

        if DEBUG:
            nc.sync.dma_start(dbg_o1[:], out1[0][:])

        # ---- BN1 all-reduce + act1 ----
        mean1, rstd1 = _bn_allreduce(nc, pools, stats1, 15.0, 1, epst,
                                     dbg=dbg_st[:, 0:4] if DEBUG else None)
        sc1, bi1 = _affine_vecs(nc, pools, g1, b1, mean1, rstd1, 15.0, 15.0, 1)

        a1_8 = pools["a1pad"].tile([C, BPC * PW], FP8, name="a1pad")
        nc.gpsimd.memset(a1_8[:], 0.0)

        # ---- phase B: act1 + conv2 per image ----
        stats2 = pools["stats"].tile([C, NTILES * 6], F32, name="stats2")
        out2 = []
        for n in range(BPC):
            o1 = out1[n]
            aff = pools["tail"].tile([C, HW], F32, name="aff", tag="tailbuf")
            nc.vector.tensor_scalar(aff[:], o1[:], sc1[:], bi1[:],
                                    op0=mybir.AluOpType.mult,
                                    op1=mybir.AluOpType.add)
            # clamp + rint -> fp8 interior of padded act1
            clp = pools["tail"].tile([C, HW], F32, name="clp", tag="tailbuf")
            nc.vector.tensor_scalar(clp[:], aff[:], 0.0, 15.0,
                                    op0=mybir.AluOpType.max, op1=mybir.AluOpType.min)
            ap8 = a1_8.rearrange("p (n h w) -> p n h w", n=BPC, w=WP)
            nc.vector.tensor_scalar(
                ap8[:, n, 1:1 + H, 1:1 + W],
                clp.rearrange("p (h w) -> p h w", w=W), C23, C23,
                op0=mybir.AluOpType.add, op1=mybir.AluOpType.subtract)
            o2 = pools["big"].tile([C, HW], F32, name=f"o2_{n}", tag="bigbuf")
            apn = a1_8[:, n * PW:(n + 1) * PW]
            _conv_image(nc, pools, apn, w2T, stats2, o2, n, "c2")
            out2.append(o2)

        if DEBUG:
            a1f = pools["xpad"].tile([C, PW], F32, name="a1f", tag="xp")
            nc.scalar.copy(a1f[:], a1_8[:, 0:PW])
            nc.sync.dma_start(dbg_a1[:], a1f[:])
            nc.sync.dma_start(dbg_o2[:], out2[0][:])

        # ---- BN2 all-reduce + tail ----
        mean2, rstd2 = _bn_allreduce(nc, pools, stats2, 225.0, 2, epst,
                                     dbg=dbg_st[:, 4:8] if DEBUG else None)
        sc2, bi2 = _affine_vecs(nc, pools, g2, b2, mean2, rstd2, 1.0, 225.0, 2)

        if DEBUG:
            nc.sync.dma_start(dbg_ab[:, 0:1], sc1[:])
            nc.sync.dma_start(dbg_ab[:, 1:2], bi1[:])
            nc.sync.dma_start(dbg_ab[:, 2:3], sc2[:])
            nc.sync.dma_start(dbg_ab[:, 3:4], bi2[:])

        for n in range(BPC):
            o2 = out2[n]
            xr = pools["xpad"].tile([C, HW], F32, name="xr", tag="xp")
            nc.sync.dma_start(xr[:], x_in[n])
            # v = o2*sc2 (ACT, exact pre-stage) ; u = (v + bi2) + x (DVE)
            v = pools["tail"].tile([C, HW], F32, name="v", tag="tailbuf")
            nc.scalar.activation(v[:], o2[:], mybir.ActivationFunctionType.Copy,
                                 scale=sc2[:])
            u = pools["tail"].tile([C, HW], F32, name="u", tag="tailbuf")
            nc.vector.scalar_tensor_tensor(u[:], v[:], bi2[:], xr[:],
                                           op0=mybir.AluOpType.add,
                                           op1=mybir.AluOpType.add)
            nc.gpsimd.tensor_scalar(u[:], u[:], 0.0, 1.0,
                                    op0=mybir.AluOpType.max, op1=mybir.AluOpType.min)
            nc.vector.tensor_scalar(u[:], u[:], 15.0, C23,
                                    op0=mybir.AluOpType.mult, op1=mybir.AluOpType.add)
            nc.vector.tensor_scalar(u[:], u[:], C23, 1.0 / 15.0,
                                    op0=mybir.AluOpType.subtract,
                                    op1=mybir.AluOpType.mult)
            nc.sync.dma_start(out_d[n], u.rearrange("p (h w) -> p h w", w=W))

    nc.compile()
    return nc


def _get_nc():
    if "nc" not in _CACHE:
        _CACHE["nc"] = build()
    return _CACHE["nc"]


def kernel(x, w1, w2, gamma1, beta1, gamma2, beta2, _trace=False):
    nc = _get_nc()
    x = np.ascontiguousarray(np.asarray(x, dtype=np.float32))
    in_common = {
        "w1": np.ascontiguousarray(np.asarray(w1, np.float32).reshape(C, C * 9)),
        "w2": np.ascontiguousarray(np.asarray(w2, np.float32).reshape(C, C * 9)),
        "gamma1": np.asarray(gamma1, np.float32).reshape(C, 1),
        "beta1": np.asarray(beta1, np.float32).reshape(C, 1),
        "gamma2": np.asarray(gamma2, np.float32).reshape(C, 1),
        "beta2": np.asarray(beta2, np.float32).reshape(C, 1),
    }
    in_maps = [dict(in_common, x=x[c * BPC:(c + 1) * BPC]) for c in range(N_CORES)]
    res = bass_utils.run_bass_kernel_spmd(nc, in_maps, core_ids=list(range(N_CORES)),
                                          trace=_trace)
    out = np.concatenate([res.results[c]["out"] for c in range(N_CORES)], axis=0)
    if _trace:
        _CACHE["last_exec_time_ns"] = res.exec_time_ns
        _CACHE["last_results"] = res
    return out


if __name__ == "__main__":
    nc = build()
    print("built ok")


# revision 60
# speedup vs baseline: 1.6006x; 1.4419x over previous
"""Trainium2 Bass kernel for quantized BasicBlock (DoReFa conv-bn-act x2 + residual).

Self-contained: builds an 8-core SPMD Bass kernel, shards the batch (64 -> 8x8),
runs via bass_utils.run_bass_kernel_spmd, gathers the full output.

Math (per core, batch shard of 8 images):
  W_int = 2*rint(tanh(w)*s + 7.5) - 15, s = 15/(2*max|tanh(w)|)   (odd ints, |.|<=15)
  conv1: S1 = conv3x3(x, W1_int)            == 15 * conv3x3(x, w_q1)
  BN1 stats of S1 over (N,H,W) all-reduced across cores; eps' = 225e-5
  act1  = clip(rint(S1*sc1 + bi1), 0, 15)   (ints 0..15, stored fp8e4m3)
  conv2: S2 = conv3x3(act1, W2_int)         == 225 * conv3x3(a_q, w_q2), exact int fp32
  BN2 stats of S2 all-reduced; eps'' = 225^2 * 1e-5
  out   = rint(15*clip(S2*sc2 + bi2 + x, 0, 1)) / 15
"""
import sys
from contextlib import ExitStack

import numpy as np

for _p in ("/opt/trn_rl_repo",):
    if _p not in sys.path:
        sys.path.append(_p)

import concourse.bass as bass
import concourse.bass_isa as bass_isa
import concourse.bacc as bacc
import concourse.mybir as mybir
import concourse.tile as tile
from concourse import bass_utils
from concourse.masks import make_identity

F32 = mybir.dt.float32
FP8 = mybir.dt.float8e4

N_CORES = 8
B, C, H, W = 64, 128, 56, 56
BPC = B // N_CORES            # images per core
HP, WP = H + 2, W + 2         # padded 58x58
PW = HP * WP                  # 3364
HW = H * W                    # 3136
RPT = 8                       # output rows per PSUM tile
TN = RPT * W                  # 448 columns per matmul
TPI = H // RPT                # 7 tiles per image
NTILES = BPC * TPI            # 56 tiles per core
N_GLOBAL = float(B * H * W)   # BN population per channel
C23 = float(2 ** 23)
EPS1 = 225.0 * 1e-5           # eps scaled for 15x conv1 output
EPS2 = 225.0 * 225.0 * 1e-5   # eps scaled for 225x conv2 output

BF16 = mybir.dt.bfloat16      # conv1 runs as two bf16 passes (x_hi + x_lo)
DEBUG = False                 # adds intermediate-dump outputs

TAPS = [(dy, dx) for dy in range(3) for dx in range(3)]

_CACHE = {}


def _quant_weights(nc, ctx, tc, pools, w_in, name):
    """DMA + DoReFa-quantize weights; returns fp32 W_int in natural (O, I*9) layout.

    All elementwise steps are in-place on one (C, C*9) tile.
    """
    wp = pools["wprep"]
    wk = wp.tile([C, C * 9], F32, name=f"{name}_wk", tag="wk")
    nc.sync.dma_start(wk[:], w_in[:])
    nc.scalar.activation(wk[:], wk[:], mybir.ActivationFunctionType.Tanh)
    # global max|tanh(w)| broadcast to all partitions
    am = wp.tile([C, 1], F32, name=f"{name}_am", tag="wam")
    nc.vector.tensor_reduce(am[:], wk[:], axis=mybir.AxisListType.X,
                            op=mybir.AluOpType.max, apply_absolute_value=True)
    amg = wp.tile([C, 1], F32, name=f"{name}_amg", tag="wamg")
    nc.gpsimd.partition_all_reduce(amg[:], am[:], channels=C,
                                   reduce_op=bass_isa.ReduceOp.max)
    s_t = wp.tile([C, 1], F32, name=f"{name}_s", tag="ws")
    nc.vector.reciprocal(s_t[:], amg[:])
    nc.vector.tensor_scalar_mul(s_t[:], s_t[:], 7.5)
    # W_int = 2*rint(tanh*s + 7.5) - 15  (rint via +2^23-2^23; 7.5 added
    # separately — 2^23+7.5 is not representable in fp32)
    nc.vector.tensor_scalar(wk[:], wk[:], s_t[:], 7.5,
                            op0=mybir.AluOpType.mult, op1=mybir.AluOpType.add)
    nc.vector.tensor_scalar(wk[:], wk[:], C23, C23,
                            op0=mybir.AluOpType.add, op1=mybir.AluOpType.subtract)
    nc.vector.tensor_scalar(wk[:], wk[:], 2.0, 15.0,
                            op0=mybir.AluOpType.mult, op1=mybir.AluOpType.subtract)
    return wk


def _transpose_taps(nc, pools, wint, identity, out_dt, name):
    """Per-tap PE transpose of W_int (O,(I,t)) -> wT (I,(t,O)) in out_dt."""
    wp = pools["wconst"]
    trp = pools["psum_tr"]
    wT = wp.tile([C, 9 * C], out_dt, name=f"{name}_T")
    wr = wint.rearrange("p (i t) -> p i t", t=9)
    for t in range(9):
        ps = trp.tile([C, C], F32, name=f"{name}_ps{t}", tag="trps")
        nc.tensor.transpose(ps[:], wr[:, :, t], identity[:])
        nc.scalar.copy(wT[:, t * C:(t + 1) * C], ps[:])
    return wT


def _conv_image(nc, pools, src_pads, wT, stats, out_sb, img_idx, name):
    """One image: 7 PSUM tiles; per tile, accumulate 9 taps x len(src_pads)
    passes (hi/lo decomposition); bn_stats + copy per tile."""
    cp = pools["psum_conv"]
    srcs = [s.rearrange("p (h w) -> p h w", w=WP) for s in src_pads]
    npass = len(srcs)
    for t in range(TPI):
        gi = img_idx * TPI + t
        ps = cp.tile([C, TN], F32, name=f"{name}_ps", tag="convps")
        for k, (dy, dx) in enumerate(TAPS):
            lhsT = wT[:, k * C:(k + 1) * C]
            for p, srcr in enumerate(srcs):
                rhs = srcr[:, RPT * t + dy: RPT * t + dy + RPT, dx: dx + W]
                nc.tensor.matmul(ps[:], lhsT, rhs,
                                 start=(k == 0 and p == 0),
                                 stop=(k == 8 and p == npass - 1))
        nc.vector.bn_stats(stats[:, gi * 6:(gi + 1) * 6], ps[:])
        nc.scalar.copy(out_sb[:, t * TN:(t + 1) * TN], ps[:])


def _warmup_allreduce_eps(nc, pools):
    """Tiny AllReduce at kernel start: warms up ncfw (first collective pays a
    large one-time latency) and produces the BN epsilon constant (8 * 1e-5/8),
    so it has a live consumer and survives DCE."""
    sp = pools["stats"]
    dp = pools["dram"]
    eps8 = sp.tile([C, 1], F32, name="eps8")
    nc.vector.memset(eps8[:], 1e-5 / N_CORES)
    cc_in = dp.tile([C, 1], F32, name="ccw_in")
    cc_out = dp.tile([C, 1], F32, name="ccw_out")
    nc.sync.dma_start(cc_in[:], eps8[:])
    nc.gpsimd.collective_compute(
        "AllReduce", mybir.AluOpType.add,
        replica_groups=[list(range(N_CORES))],
        ins=[cc_in.opt()], outs=[cc_out.opt()],
    )
    epst = sp.tile([C, 1], F32, name="epst")
    nc.sync.dma_start(epst[:], cc_out[:])
    return epst


def _bn_allreduce(nc, pools, stats, k_scale, cc_idx, epst, dbg=None):
    """bn_aggr local stats (of k*S) -> unscaled (mean, E[x^2])/8 payload ->
    AllReduce -> mean_u, rstd_u in UNSCALED units (good Sqrt-LUT range)."""
    sp = pools["stats"]
    dp = pools["dram"]
    loc = sp.tile([C, 2], F32, name=f"bn{cc_idx}_loc")
    nc.vector.bn_aggr(loc[:], stats.rearrange("p (t k) -> p t k", k=3))
    # rescale to unscaled units: mean /= k, var /= k^2
    nc.vector.tensor_scalar(loc[:, 0:1], loc[:, 0:1], 1.0 / k_scale, None,
                            op0=mybir.AluOpType.mult)
    nc.vector.tensor_scalar(loc[:, 1:2], loc[:, 1:2], 1.0 / (k_scale * k_scale),
                            None, op0=mybir.AluOpType.mult)
    if dbg is not None:
        nc.sync.dma_start(dbg[:, 0:2], loc[:])
    pay = sp.tile([C, 2], F32, name=f"bn{cc_idx}_pay")
    # pay0 = mean/8 ; pay1 = (var + mean^2)/8
    nc.vector.tensor_scalar_mul(pay[:, 0:1], loc[:, 0:1], 1.0 / N_CORES)
    msq = sp.tile([C, 1], F32, name=f"bn{cc_idx}_msq")
    nc.vector.scalar_tensor_tensor(msq[:], loc[:, 0:1], 1.0 / N_CORES, loc[:, 0:1],
                                   op0=mybir.AluOpType.mult, op1=mybir.AluOpType.mult)
    nc.vector.scalar_tensor_tensor(pay[:, 1:2], loc[:, 1:2], 1.0 / N_CORES, msq[:],
                                   op0=mybir.AluOpType.mult, op1=mybir.AluOpType.add)
    cc_in = dp.tile([C, 2], F32, name=f"cc{cc_idx}_in")
    cc_out = dp.tile([C, 2], F32, name=f"cc{cc_idx}_out")
    nc.sync.dma_start(cc_in[:], pay[:])
    nc.gpsimd.collective_compute(
        "AllReduce", mybir.AluOpType.add,
        replica_groups=[list(range(N_CORES))],
        ins=[cc_in.opt()], outs=[cc_out.opt()],
    )
    gs = sp.tile([C, 2], F32, name=f"bn{cc_idx}_gs")
    nc.sync.dma_start(gs[:], cc_out[:])
    if dbg is not None:
        nc.sync.dma_start(dbg[:, 2:4], gs[:])
    mean_g = gs[:, 0:1]
    # var_g = E[x^2] - mean^2 ; rstd = 1/sqrt(var_g + eps)
    m2 = sp.tile([C, 1], F32, name=f"bn{cc_idx}_m2")
    nc.vector.scalar_tensor_tensor(m2[:], mean_g, 1.0, mean_g,
                                   op0=mybir.AluOpType.mult, op1=mybir.AluOpType.mult)
    varg = sp.tile([C, 1], F32, name=f"bn{cc_idx}_var")
    nc.vector.scalar_tensor_tensor(varg[:], m2[:], -1.0, gs[:, 1:2],
                                   op0=mybir.AluOpType.mult, op1=mybir.AluOpType.add)
    std = sp.tile([C, 1], F32, name=f"bn{cc_idx}_std")
    nc.scalar.activation(std[:], varg[:], mybir.ActivationFunctionType.Sqrt,
                         bias=epst[:])
    rstd = sp.tile([C, 1], F32, name=f"bn{cc_idx}_rstd")
    nc.vector.reciprocal(rstd[:], std[:])
    return mean_g, rstd


def _affine_vecs(nc, pools, gamma, beta, mean_u, rstd_u, m_out, k_scale, cc_idx):
    """For y_out = m*bn(S/k): sc = m*gamma*rstd/k ; bi = m*(beta - mean_u*gamma*rstd).

    mean_u / rstd_u are in unscaled units; S is the k-scaled PSUM value.
    """
    sp = pools["stats"]
    gr = sp.tile([C, 1], F32, name=f"gr{cc_idx}")
    nc.vector.scalar_tensor_tensor(gr[:], gamma[:], 1.0, rstd_u[:],
                                   op0=mybir.AluOpType.bypass,
                                   op1=mybir.AluOpType.mult)
    sc = sp.tile([C, 1], F32, name=f"sc{cc_idx}")
    nc.vector.tensor_scalar_mul(sc[:], gr[:], m_out / k_scale)
    negms = sp.tile([C, 1], F32, name=f"negms{cc_idx}")
    nc.vector.scalar_tensor_tensor(negms[:], mean_u, -1.0, gr[:],
                                   op0=mybir.AluOpType.mult, op1=mybir.AluOpType.mult)
    bi = sp.tile([C, 1], F32, name=f"bi{cc_idx}")
    nc.vector.scalar_tensor_tensor(bi[:], negms[:], 1.0, beta[:],
                                   op0=mybir.AluOpType.bypass,
                                   op1=mybir.AluOpType.add)
    # bi currently = (-mean*gr) + beta ; scale by m
    nc.vector.tensor_scalar_mul(bi[:], bi[:], m_out)
    return sc, bi


def build():
    nc = bacc.Bacc("TRN2", target_bir_lowering=False, debug=False,
                   enable_asserts=False, num_devices=N_CORES)
    x_in = nc.dram_tensor("x", [BPC, C, H, W], F32, kind="ExternalInput").ap()
    w1_in = nc.dram_tensor("w1", [C, C * 9], F32, kind="ExternalInput").ap()
    w2_in = nc.dram_tensor("w2", [C, C * 9], F32, kind="ExternalInput").ap()
    g1_in = nc.dram_tensor("gamma1", [C, 1], F32, kind="ExternalInput").ap()
    b1_in = nc.dram_tensor("beta1", [C, 1], F32, kind="ExternalInput").ap()
    g2_in = nc.dram_tensor("gamma2", [C, 1], F32, kind="ExternalInput").ap()
    b2_in = nc.dram_tensor("beta2", [C, 1], F32, kind="ExternalInput").ap()
    out_d = nc.dram_tensor("out", [BPC, C, H, W], F32, kind="ExternalOutput").ap()
    if DEBUG:
        dbg_w1 = nc.dram_tensor("dbg_w1", [C, C * 9], F32, kind="ExternalOutput").ap()
        dbg_o1 = nc.dram_tensor("dbg_o1", [C, HW], F32, kind="ExternalOutput").ap()
        dbg_st = nc.dram_tensor("dbg_st", [C, 8], F32, kind="ExternalOutput").ap()
        dbg_a1 = nc.dram_tensor("dbg_a1", [C, PW], F32, kind="ExternalOutput").ap()
        dbg_o2 = nc.dram_tensor("dbg_o2", [C, HW], F32, kind="ExternalOutput").ap()
        dbg_ab = nc.dram_tensor("dbg_ab", [C, 4], F32, kind="ExternalOutput").ap()

    with tile.TileContext(nc) as tc, ExitStack() as ctx:
        pools = {
            "wprep": ctx.enter_context(tc.tile_pool(name="wprep", bufs=1)),
            "wconst": ctx.enter_context(tc.tile_pool(name="wconst", bufs=1)),
            "stats": ctx.enter_context(tc.tile_pool(name="stats", bufs=1)),
            "big": ctx.enter_context(tc.tile_pool(name="big", bufs=8)),
            "xpart": ctx.enter_context(tc.tile_pool(name="xpart", bufs=4)),
            "a1pad": ctx.enter_context(tc.tile_pool(name="a1pad", bufs=1)),
            "tail": ctx.enter_context(tc.tile_pool(name="tail", bufs=2)),
            "psum_conv": ctx.enter_context(
                tc.tile_pool(name="psum_conv", bufs=7, space="PSUM")),
            "psum_tr": ctx.enter_context(
                tc.tile_pool(name="psum_tr", bufs=1, space="PSUM")),
            "dram": ctx.enter_context(tc.tile_pool(name="dram", bufs=4, space="DRAM")),
        }
        consts = pools["wconst"]

        # per-channel params
        g1 = consts.tile([C, 1], F32, name="g1"); nc.sync.dma_start(g1[:], g1_in[:])
        b1 = consts.tile([C, 1], F32, name="b1"); nc.sync.dma_start(b1[:], b1_in[:])
        g2 = consts.tile([C, 1], F32, name="g2"); nc.sync.dma_start(g2[:], g2_in[:])
        b2 = consts.tile([C, 1], F32, name="b2"); nc.sync.dma_start(b2[:], b2_in[:])

        identity = consts.tile([C, C], F32, name="identity")
        make_identity(nc, identity[:])

        epst = _warmup_allreduce_eps(nc, pools)

        # ---- weights ----
        w1i = _quant_weights(nc, ctx, tc, pools, w1_in, "w1")
        if DEBUG:
            nc.sync.dma_start(dbg_w1[:], w1i[:])
        w1T = _transpose_taps(nc, pools, w1i, identity, BF16, "w1")
        w2i = _quant_weights(nc, ctx, tc, pools, w2_in, "w2")
        w2T = _transpose_taps(nc, pools, w2i, identity, FP8, "w2")

        # ---- phase A: conv1 per image (bf16 hi/lo two-pass) ----
        stats1 = pools["stats"].tile([C, NTILES * 6], F32, name="stats1")
        out1 = []
        for n in range(BPC):
            xh = pools["xpart"].tile([C, PW], BF16, name="xh", tag="xpart")
            xl = pools["xpart"].tile([C, PW], BF16, name="xl", tag="xpart")
            for xb in (xh, xl):
                xbr = xb.rearrange("p (h w) -> p h w", w=WP)
                nc.vector.memset(xbr[:, 0, :], 0.0)
                nc.vector.memset(xbr[:, HP - 1, :], 0.0)
                side = xb[:, WP - 1:WP - 1 + (HP - 1) * WP].rearrange(
                    "p (a b) -> p a b", b=WP)
                nc.vector.memset(side[:, :, 0:2], 0.0)
            xs = pools["tail"].tile([C, HW], F32, name="xs", tag="tailbuf")
            nc.sync.dma_start(xs[:], x_in[n])
            xsr = xs.rearrange("p (h w) -> p h w", w=W)
            xhr = xh.rearrange("p (h w) -> p h w", w=WP)
            xlr = xl.rearrange("p (h w) -> p h w", w=WP)
            # x_hi = bf16(x) on ACT ; x_lo = bf16(x - x_hi) on DVE
            nc.scalar.copy(xhr[:, 1:1 + H, 1:1 + W], xsr[:])
            nc.vector.scalar_tensor_tensor(
                xlr[:, 1:1 + H, 1:1 + W], xsr[:], 1.0,
                xhr[:, 1:1 + H, 1:1 + W],
                op0=mybir.AluOpType.bypass, op1=mybir.AluOpType.subtract)
            o1 = pools["big"].tile([C, HW], F32, name=f"o1_{n}", tag="bigbuf")
            _conv_image(nc, pools, [xh, xl], w1T, stats1, o1, n, "c1")
            out1.append(o1)

        if DEBUG:
            nc.sync.dma_start(dbg_o1[:], out1[0][:])

        # ---- BN1 all-reduce + act1 ----
        mean1, rstd1 = _bn_allreduce(nc, pools, stats1, 15.0, 1, epst,
                                     dbg=dbg_st[:, 0:4] if DEBUG else None)
        sc1, bi1 = _affine_vecs(nc, pools, g1, b1, mean1, rstd1, 15.0, 15.0, 1)

        a1_8 = pools["a1pad"].tile([C, BPC * PW], FP8, name="a1pad")
        ap8 = a1_8.rearrange("p (n h w) -> p n h w", n=BPC, w=WP)
        for n in range(BPC):
            # halo-only zeroing of act1 padding (fp8 memsets)
            nc.vector.memset(ap8[:, n, 0, :], 0.0)
            nc.vector.memset(ap8[:, n, HP - 1, :], 0.0)
            aside = a1_8[:, n * PW + WP - 1:n * PW + WP - 1 + (HP - 1) * WP]
            nc.vector.memset(
                aside.rearrange("p (a b) -> p a b", b=WP)[:, :, 0:2], 0.0)

        # ---- phase B: act1 (in-place on out1) + conv2 per image ----
        stats2 = pools["stats"].tile([C, NTILES * 6], F32, name="stats2")
        out2 = []
        for n in range(BPC):
            o1 = out1[n]
            nc.vector.tensor_scalar(o1[:], o1[:], sc1[:], bi1[:],
                                    op0=mybir.AluOpType.mult,
                                    op1=mybir.AluOpType.add)
            nc.vector.tensor_scalar(o1[:], o1[:], 0.0, 15.0,
                                    op0=mybir.AluOpType.max, op1=mybir.AluOpType.min)
            nc.vector.tensor_scalar(
                ap8[:, n, 1:1 + H, 1:1 + W],
                o1.rearrange("p (h w) -> p h w", w=W), C23, C23,
                op0=mybir.AluOpType.add, op1=mybir.AluOpType.subtract)
            o2 = pools["big"].tile([C, HW], F32, name=f"o2_{n}", tag="bigbuf")
            apn = a1_8[:, n * PW:(n + 1) * PW]
            _conv_image(nc, pools, [apn], w2T, stats2, o2, n, "c2")
            out2.append(o2)

        if DEBUG:
            a1f = pools["stats"].tile([C, PW], F32, name="a1f")
            nc.scalar.copy(a1f[:], a1_8[:, 0:PW])
            nc.sync.dma_start(dbg_a1[:], a1f[:])
            nc.sync.dma_start(dbg_o2[:], out2[0][:])

        # ---- BN2 all-reduce + tail ----
        mean2, rstd2 = _bn_allreduce(nc, pools, stats2, 225.0, 2, epst,
                                     dbg=dbg_st[:, 4:8] if DEBUG else None)
        sc2, bi2 = _affine_vecs(nc, pools, g2, b2, mean2, rstd2, 1.0, 225.0, 2)

        if DEBUG:
            nc.sync.dma_start(dbg_ab[:, 0:1], sc1[:])
            nc.sync.dma_start(dbg_ab[:, 1:2], bi1[:])
            nc.sync.dma_start(dbg_ab[:, 2:3], sc2[:])
            nc.sync.dma_start(dbg_ab[:, 3:4], bi2[:])

        HWH = HW // 2
        for n in range(BPC):
            o2 = out2[n]
            # residual x in two halves (reuses the bf16 xpart slots)
            xra = pools["xpart"].tile([C, HWH], F32, name="xra", tag="xpart")
            xrb = pools["xpart"].tile([C, HWH], F32, name="xrb", tag="xpart")
            xf = x_in[n].rearrange("c h w -> c (h w)")
            nc.sync.dma_start(xra[:], xf[:, 0:HWH])
            nc.sync.dma_start(xrb[:], xf[:, HWH:HW])
            # in-place tail on o2: v=o2*sc2 (ACT) ; +bi2+x ; clamp ; rint*15 ;
            # /15 — all exact ALU ops
            nc.scalar.activation(o2[:], o2[:], mybir.ActivationFunctionType.Copy,
                                 scale=sc2[:])
            nc.vector.scalar_tensor_tensor(o2[:, 0:HWH], o2[:, 0:HWH], bi2[:],
                                           xra[:], op0=mybir.AluOpType.add,
                                           op1=mybir.AluOpType.add)
            nc.vector.scalar_tensor_tensor(o2[:, HWH:HW], o2[:, HWH:HW], bi2[:],
                                           xrb[:], op0=mybir.AluOpType.add,
                                           op1=mybir.AluOpType.add)
            nc.vector.tensor_scalar(o2[:], o2[:], 0.0, 1.0,
                                    op0=mybir.AluOpType.max, op1=mybir.AluOpType.min)
            nc.scalar.activation(o2[:], o2[:], mybir.ActivationFunctionType.Copy,
                                 scale=15.0, bias=C23)
            nc.vector.tensor_scalar(o2[:], o2[:], C23, 1.0 / 15.0,
                                    op0=mybir.AluOpType.subtract,
                                    op1=mybir.AluOpType.mult)
            nc.sync.dma_start(out_d[n], o2.rearrange("p (h w) -> p h w", w=W))

    nc.compile()
    return nc


def _get_nc():
    if "nc" not in _CACHE:
        _CACHE["nc"] = build()
    return _CACHE["nc"]


def kernel(x, w1, w2, gamma1, beta1, gamma2, beta2, _trace=False):
    nc = _get_nc()
    x = np.ascontiguousarray(np.asarray(x, dtype=np.float32))
    in_common = {
        "w1": np.ascontiguousarray(np.asarray(w1, np.float32).reshape(C, C * 9)),
        "w2": np.ascontiguousarray(np.asarray(w2, np.float32).reshape(C, C * 9)),
        "gamma1": np.asarray(gamma1, np.float32).reshape(C, 1),
        "beta1": np.asarray(beta1, np.float32).reshape(C, 1),
        "gamma2": np.asarray(gamma2, np.float32).reshape(C, 1),
        "beta2": np.asarray(beta2, np.float32).reshape(C, 1),
    }
    in_maps = [dict(in_common, x=x[c * BPC:(c + 1) * BPC]) for c in range(N_CORES)]
    res = bass_utils.run_bass_kernel_spmd(nc, in_maps, core_ids=list(range(N_CORES)),
                                          trace=_trace)
    out = np.concatenate([res.results[c]["out"] for c in range(N_CORES)], axis=0)
    if _trace:
        _CACHE["last_exec_time_ns"] = res.exec_time_ns
        _CACHE["last_results"] = res
    return out


if __name__ == "__main__":
    nc = build()
    print("built ok")


# revision 63
# speedup vs baseline: 1.6798x; 1.0495x over previous
"""Trainium2 Bass kernel for quantized BasicBlock (DoReFa conv-bn-act x2 + residual).

Self-contained: builds an 8-core SPMD Bass kernel, shards the batch (64 -> 8x8),
runs via bass_utils.run_bass_kernel_spmd, gathers the full output.

Math (per core, batch shard of 8 images):
  W_int = 2*rint(tanh(w)*s + 7.5) - 15, s = 15/(2*max|tanh(w)|)   (odd ints, |.|<=15)
  conv1: S1 = conv3x3(x, W1_int)            == 15 * conv3x3(x, w_q1)
  BN1 stats of S1 over (N,H,W) all-reduced across cores; eps' = 225e-5
  act1  = clip(rint(S1*sc1 + bi1), 0, 15)   (ints 0..15, stored fp8e4m3)
  conv2: S2 = conv3x3(act1, W2_int)         == 225 * conv3x3(a_q, w_q2), exact int fp32
  BN2 stats of S2 all-reduced; eps'' = 225^2 * 1e-5
  out   = rint(15*clip(S2*sc2 + bi2 + x, 0, 1)) / 15
"""
import sys
from contextlib import ExitStack

import numpy as np

for _p in ("/opt/trn_rl_repo",):
    if _p not in sys.path:
        sys.path.append(_p)

import concourse.bass as bass
import concourse.bass_isa as bass_isa
import concourse.bacc as bacc
import concourse.mybir as mybir
import concourse.tile as tile
from concourse import bass_utils
from concourse.masks import make_identity

F32 = mybir.dt.float32
FP8 = mybir.dt.float8e4

N_CORES = 8
B, C, H, W = 64, 128, 56, 56
BPC = B // N_CORES            # images per core
HP, WP = H + 2, W + 2         # padded 58x58
PW = HP * WP                  # 3364
HW = H * W                    # 3136
RPT = 8                       # output rows per PSUM tile
TN = RPT * W                  # 448 columns per matmul
TPI = H // RPT                # 7 tiles per image
NTILES = BPC * TPI            # 56 tiles per core
N_GLOBAL = float(B * H * W)   # BN population per channel
C23 = float(2 ** 23)
EPS1 = 225.0 * 1e-5           # eps scaled for 15x conv1 output
EPS2 = 225.0 * 225.0 * 1e-5   # eps scaled for 225x conv2 output

BF16 = mybir.dt.bfloat16      # conv1 runs as two bf16 passes (x_hi + x_lo)
DEBUG = False                 # adds intermediate-dump outputs

TAPS = [(dy, dx) for dy in range(3) for dx in range(3)]

_CACHE = {}


def _quant_weights(nc, ctx, tc, pools, w_in, name):
    """DMA + DoReFa-quantize weights; returns fp32 W_int in natural (O, I*9) layout.

    All elementwise steps are in-place on one (C, C*9) tile.
    """
    wp = pools["wprep"]
    wk = wp.tile([C, C * 9], F32, name=f"{name}_wk", tag=f"wk_{name}")
    # split the load across two DMA queues; absmax runs on the raw weights in
    # parallel with tanh (max|tanh(w)| == tanh(max|w|))
    half = C * 9 // 2
    nc.sync.dma_start(wk[:, 0:half], w_in[:, 0:half])
    nc.scalar.dma_start(wk[:, half:], w_in[:, half:])
    am = wp.tile([C, 1], F32, name=f"{name}_am", tag="wam")
    nc.vector.tensor_reduce(am[:], wk[:], axis=mybir.AxisListType.X,
                            op=mybir.AluOpType.max, apply_absolute_value=True)
    nc.scalar.activation(wk[:], wk[:], mybir.ActivationFunctionType.Tanh)
    amg = wp.tile([C, 1], F32, name=f"{name}_amg", tag="wamg")
    nc.gpsimd.partition_all_reduce(amg[:], am[:], channels=C,
                                   reduce_op=bass_isa.ReduceOp.max)
    s_t = wp.tile([C, 1], F32, name=f"{name}_s", tag="ws")
    nc.scalar.activation(s_t[:], amg[:], mybir.ActivationFunctionType.Tanh)
    nc.vector.reciprocal(s_t[:], s_t[:])
    nc.vector.tensor_scalar_mul(s_t[:], s_t[:], 7.5)
    # W_int = 2*rint(tanh*s + 7.5) - 15  (rint via +2^23-2^23; 7.5 added
    # separately — 2^23+7.5 is not representable in fp32)
    nc.vector.tensor_scalar(wk[:], wk[:], s_t[:], 7.5,
                            op0=mybir.AluOpType.mult, op1=mybir.AluOpType.add)
    nc.vector.tensor_scalar(wk[:], wk[:], C23, C23,
                            op0=mybir.AluOpType.add, op1=mybir.AluOpType.subtract)
    nc.vector.tensor_scalar(wk[:], wk[:], 2.0, 15.0,
                            op0=mybir.AluOpType.mult, op1=mybir.AluOpType.subtract)
    return wk


def _transpose_taps(nc, pools, wint, identity, out_dt, name):
    """Per-tap PE transpose of W_int (O,(I,t)) -> wT (I,(t,O)) in out_dt."""
    wp = pools["wconst"]
    trp = pools["psum_tr"]
    wT = wp.tile([C, 9 * C], out_dt, name=f"{name}_T")
    wr = wint.rearrange("p (i t) -> p i t", t=9)
    for t in range(9):
        ps = trp.tile([C, C], F32, name=f"{name}_ps{t}", tag="trps")
        nc.tensor.transpose(ps[:], wr[:, :, t], identity[:])
        nc.scalar.copy(wT[:, t * C:(t + 1) * C], ps[:])
    return wT


def _conv_image(nc, pools, src_pads, wT, stats, out_sb, img_idx, name):
    """One image: 7 PSUM tiles; per tile, accumulate 9 taps x len(src_pads)
    passes (hi/lo decomposition); bn_stats + copy per tile."""
    cp = pools["psum_conv"]
    srcs = [s.rearrange("p (h w) -> p h w", w=WP) for s in src_pads]
    npass = len(srcs)
    for t in range(TPI):
        gi = img_idx * TPI + t
        ps = cp.tile([C, TN], F32, name=f"{name}_ps", tag="convps")
        for k, (dy, dx) in enumerate(TAPS):
            lhsT = wT[:, k * C:(k + 1) * C]
            for p, srcr in enumerate(srcs):
                rhs = srcr[:, RPT * t + dy: RPT * t + dy + RPT, dx: dx + W]
                nc.tensor.matmul(ps[:], lhsT, rhs,
                                 start=(k == 0 and p == 0),
                                 stop=(k == 8 and p == npass - 1))
        nc.vector.bn_stats(stats[:, gi * 6:(gi + 1) * 6], ps[:])
        nc.scalar.copy(out_sb[:, t * TN:(t + 1) * TN], ps[:])


def _warmup_allreduce_eps(nc, pools):
    """Tiny AllReduce at kernel start: warms up ncfw (first collective pays a
    large one-time latency) and produces the BN epsilon constant (8 * 1e-5/8),
    so it has a live consumer and survives DCE."""
    sp = pools["stats"]
    dp = pools["dram"]
    eps8 = sp.tile([C, 1], F32, name="eps8")
    nc.vector.memset(eps8[:], 1e-5 / N_CORES)
    cc_in = dp.tile([C, 1], F32, name="ccw_in")
    cc_out = dp.tile([C, 1], F32, name="ccw_out")
    nc.sync.dma_start(cc_in[:], eps8[:])
    nc.gpsimd.collective_compute(
        "AllReduce", mybir.AluOpType.add,
        replica_groups=[list(range(N_CORES))],
        ins=[cc_in.opt()], outs=[cc_out.opt()],
    )
    epst = sp.tile([C, 1], F32, name="epst")
    nc.sync.dma_start(epst[:], cc_out[:])
    return epst


def _bn_allreduce(nc, pools, stats, k_scale, cc_idx, epst, dbg=None):
    """bn_aggr local stats (of k*S) -> unscaled (mean, E[x^2])/8 payload ->
    AllReduce -> mean_u, rstd_u in UNSCALED units (good Sqrt-LUT range)."""
    sp = pools["stats"]
    dp = pools["dram"]
    loc = sp.tile([C, 2], F32, name=f"bn{cc_idx}_loc")
    nc.vector.bn_aggr(loc[:], stats.rearrange("p (t k) -> p t k", k=3))
    # rescale to unscaled units: mean /= k, var /= k^2
    nc.vector.tensor_scalar(loc[:, 0:1], loc[:, 0:1], 1.0 / k_scale, None,
                            op0=mybir.AluOpType.mult)
    nc.vector.tensor_scalar(loc[:, 1:2], loc[:, 1:2], 1.0 / (k_scale * k_scale),
                            None, op0=mybir.AluOpType.mult)
    if dbg is not None:
        nc.sync.dma_start(dbg[:, 0:2], loc[:])
    pay = sp.tile([C, 2], F32, name=f"bn{cc_idx}_pay")
    # pay0 = mean/8 ; pay1 = (var + mean^2)/8
    nc.vector.tensor_scalar_mul(pay[:, 0:1], loc[:, 0:1], 1.0 / N_CORES)
    msq = sp.tile([C, 1], F32, name=f"bn{cc_idx}_msq")
    nc.vector.scalar_tensor_tensor(msq[:], loc[:, 0:1], 1.0 / N_CORES, loc[:, 0:1],
                                   op0=mybir.AluOpType.mult, op1=mybir.AluOpType.mult)
    nc.vector.scalar_tensor_tensor(pay[:, 1:2], loc[:, 1:2], 1.0 / N_CORES, msq[:],
                                   op0=mybir.AluOpType.mult, op1=mybir.AluOpType.add)
    cc_in = dp.tile([C, 2], F32, name=f"cc{cc_idx}_in")
    cc_out = dp.tile([C, 2], F32, name=f"cc{cc_idx}_out")
    nc.sync.dma_start(cc_in[:], pay[:])
    nc.gpsimd.collective_compute(
        "AllReduce", mybir.AluOpType.add,
        replica_groups=[list(range(N_CORES))],
        ins=[cc_in.opt()], outs=[cc_out.opt()],
    )
    gs = sp.tile([C, 2], F32, name=f"bn{cc_idx}_gs")
    nc.sync.dma_start(gs[:], cc_out[:])
    if dbg is not None:
        nc.sync.dma_start(dbg[:, 2:4], gs[:])
    mean_g = gs[:, 0:1]
    # var_g = E[x^2] - mean^2 ; rstd = 1/sqrt(var_g + eps)
    m2 = sp.tile([C, 1], F32, name=f"bn{cc_idx}_m2")
    nc.vector.scalar_tensor_tensor(m2[:], mean_g, 1.0, mean_g,
                                   op0=mybir.AluOpType.mult, op1=mybir.AluOpType.mult)
    varg = sp.tile([C, 1], F32, name=f"bn{cc_idx}_var")
    nc.vector.scalar_tensor_tensor(varg[:], m2[:], -1.0, gs[:, 1:2],
                                   op0=mybir.AluOpType.mult, op1=mybir.AluOpType.add)
    std = sp.tile([C, 1], F32, name=f"bn{cc_idx}_std")
    nc.scalar.activation(std[:], varg[:], mybir.ActivationFunctionType.Sqrt,
                         bias=epst[:])
    rstd = sp.tile([C, 1], F32, name=f"bn{cc_idx}_rstd")
    nc.vector.reciprocal(rstd[:], std[:])
    return mean_g, rstd


def _affine_vecs(nc, pools, gamma, beta, mean_u, rstd_u, m_out, k_scale, cc_idx):
    """For y_out = m*bn(S/k): sc = m*gamma*rstd/k ; bi = m*(beta - mean_u*gamma*rstd).

    mean_u / rstd_u are in unscaled units; S is the k-scaled PSUM value.
    """
    sp = pools["stats"]
    gr = sp.tile([C, 1], F32, name=f"gr{cc_idx}")
    nc.vector.scalar_tensor_tensor(gr[:], gamma[:], 1.0, rstd_u[:],
                                   op0=mybir.AluOpType.bypass,
                                   op1=mybir.AluOpType.mult)
    sc = sp.tile([C, 1], F32, name=f"sc{cc_idx}")
    nc.vector.tensor_scalar_mul(sc[:], gr[:], m_out / k_scale)
    negms = sp.tile([C, 1], F32, name=f"negms{cc_idx}")
    nc.vector.scalar_tensor_tensor(negms[:], mean_u, -1.0, gr[:],
                                   op0=mybir.AluOpType.mult, op1=mybir.AluOpType.mult)
    bi = sp.tile([C, 1], F32, name=f"bi{cc_idx}")
    nc.vector.scalar_tensor_tensor(bi[:], negms[:], 1.0, beta[:],
                                   op0=mybir.AluOpType.bypass,
                                   op1=mybir.AluOpType.add)
    # bi currently = (-mean*gr) + beta ; scale by m
    nc.vector.tensor_scalar_mul(bi[:], bi[:], m_out)
    return sc, bi


def build():
    nc = bacc.Bacc("TRN2", target_bir_lowering=False, debug=False,
                   enable_asserts=False, num_devices=N_CORES)
    x_in = nc.dram_tensor("x", [BPC, C, H, W], F32, kind="ExternalInput").ap()
    w1_in = nc.dram_tensor("w1", [C, C * 9], F32, kind="ExternalInput").ap()
    w2_in = nc.dram_tensor("w2", [C, C * 9], F32, kind="ExternalInput").ap()
    g1_in = nc.dram_tensor("gamma1", [C, 1], F32, kind="ExternalInput").ap()
    b1_in = nc.dram_tensor("beta1", [C, 1], F32, kind="ExternalInput").ap()
    g2_in = nc.dram_tensor("gamma2", [C, 1], F32, kind="ExternalInput").ap()
    b2_in = nc.dram_tensor("beta2", [C, 1], F32, kind="ExternalInput").ap()
    out_d = nc.dram_tensor("out", [BPC, C, H, W], F32, kind="ExternalOutput").ap()
    if DEBUG:
        dbg_w1 = nc.dram_tensor("dbg_w1", [C, C * 9], F32, kind="ExternalOutput").ap()
        dbg_o1 = nc.dram_tensor("dbg_o1", [C, HW], F32, kind="ExternalOutput").ap()
        dbg_st = nc.dram_tensor("dbg_st", [C, 8], F32, kind="ExternalOutput").ap()
        dbg_a1 = nc.dram_tensor("dbg_a1", [C, PW], F32, kind="ExternalOutput").ap()
        dbg_o2 = nc.dram_tensor("dbg_o2", [C, HW], F32, kind="ExternalOutput").ap()
        dbg_ab = nc.dram_tensor("dbg_ab", [C, 4], F32, kind="ExternalOutput").ap()

    with tile.TileContext(nc) as tc, ExitStack() as ctx:
        pools = {
            "wprep": ctx.enter_context(tc.tile_pool(name="wprep", bufs=1)),
            "wconst": ctx.enter_context(tc.tile_pool(name="wconst", bufs=1)),
            "stats": ctx.enter_context(tc.tile_pool(name="stats", bufs=1)),
            "big": ctx.enter_context(tc.tile_pool(name="big", bufs=8)),
            "xpart": ctx.enter_context(tc.tile_pool(name="xpart", bufs=4)),
            "a1pad": ctx.enter_context(tc.tile_pool(name="a1pad", bufs=1)),
            "tail": ctx.enter_context(tc.tile_pool(name="tail", bufs=2)),
            "psum_conv": ctx.enter_context(
                tc.tile_pool(name="psum_conv", bufs=7, space="PSUM")),
            "psum_tr": ctx.enter_context(
                tc.tile_pool(name="psum_tr", bufs=1, space="PSUM")),
            "dram": ctx.enter_context(tc.tile_pool(name="dram", bufs=4, space="DRAM")),
        }
        consts = pools["wconst"]

        # per-channel params
        g1 = consts.tile([C, 1], F32, name="g1"); nc.sync.dma_start(g1[:], g1_in[:])
        b1 = consts.tile([C, 1], F32, name="b1"); nc.sync.dma_start(b1[:], b1_in[:])
        g2 = consts.tile([C, 1], F32, name="g2"); nc.sync.dma_start(g2[:], g2_in[:])
        b2 = consts.tile([C, 1], F32, name="b2"); nc.sync.dma_start(b2[:], b2_in[:])

        identity = consts.tile([C, C], F32, name="identity")
        make_identity(nc, identity[:])

        epst = _warmup_allreduce_eps(nc, pools)

        # ---- weights ----
        w1i = _quant_weights(nc, ctx, tc, pools, w1_in, "w1")
        if DEBUG:
            nc.sync.dma_start(dbg_w1[:], w1i[:])
        w1T = _transpose_taps(nc, pools, w1i, identity, BF16, "w1")
        w2i = _quant_weights(nc, ctx, tc, pools, w2_in, "w2")
        w2T = _transpose_taps(nc, pools, w2i, identity, FP8, "w2")

        # ---- phase A: conv1 per image (bf16 hi/lo two-pass) ----
        stats1 = pools["stats"].tile([C, NTILES * 6], F32, name="stats1")
        out1 = []
        for n in range(BPC):
            xh = pools["xpart"].tile([C, PW], BF16, name="xh", tag="xpart")
            xl = pools["xpart"].tile([C, PW], BF16, name="xl", tag="xpart")
            for xb in (xh, xl):
                xbr = xb.rearrange("p (h w) -> p h w", w=WP)
                nc.vector.memset(xbr[:, 0, :], 0.0)
                nc.vector.memset(xbr[:, HP - 1, :], 0.0)
                side = xb[:, WP - 1:WP - 1 + (HP - 1) * WP].rearrange(
                    "p (a b) -> p a b", b=WP)
                nc.vector.memset(side[:, :, 0:2], 0.0)
            xs = pools["tail"].tile([C, HW], F32, name="xs", tag="tailbuf")
            nc.sync.dma_start(xs[:], x_in[n])
            xsr = xs.rearrange("p (h w) -> p h w", w=W)
            xhr = xh.rearrange("p (h w) -> p h w", w=WP)
            xlr = xl.rearrange("p (h w) -> p h w", w=WP)
            # x_hi = bf16(x) on ACT ; x_lo = bf16(x - x_hi) on DVE
            nc.scalar.copy(xhr[:, 1:1 + H, 1:1 + W], xsr[:])
            nc.vector.scalar_tensor_tensor(
                xlr[:, 1:1 + H, 1:1 + W], xsr[:], 1.0,
                xhr[:, 1:1 + H, 1:1 + W],
                op0=mybir.AluOpType.bypass, op1=mybir.AluOpType.subtract)
            o1 = pools["big"].tile([C, HW], F32, name=f"o1_{n}", tag="bigbuf")
            _conv_image(nc, pools, [xh, xl], w1T, stats1, o1, n, "c1")
            out1.append(o1)

        if DEBUG:
            nc.sync.dma_start(dbg_o1[:], out1[0][:])

        # ---- BN1 all-reduce + act1 ----
        mean1, rstd1 = _bn_allreduce(nc, pools, stats1, 15.0, 1, epst,
                                     dbg=dbg_st[:, 0:4] if DEBUG else None)
        sc1, bi1 = _affine_vecs(nc, pools, g1, b1, mean1, rstd1, 15.0, 15.0, 1)

        a1_8 = pools["a1pad"].tile([C, BPC * PW], FP8, name="a1pad")
        ap8 = a1_8.rearrange("p (n h w) -> p n h w", n=BPC, w=WP)
        for n in range(BPC):
            # halo-only zeroing of act1 padding (fp8 memsets)
            nc.vector.memset(ap8[:, n, 0, :], 0.0)
            nc.vector.memset(ap8[:, n, HP - 1, :], 0.0)
            aside = a1_8[:, n * PW + WP - 1:n * PW + WP - 1 + (HP - 1) * WP]
            nc.vector.memset(
                aside.rearrange("p (a b) -> p a b", b=WP)[:, :, 0:2], 0.0)

        # ---- phase B: act1 (in-place on out1) + conv2 per image ----
        stats2 = pools["stats"].tile([C, NTILES * 6], F32, name="stats2")
        out2 = []
        for n in range(BPC):
            o1 = out1[n]
            nc.vector.tensor_scalar(o1[:], o1[:], sc1[:], bi1[:],
                                    op0=mybir.AluOpType.mult,
                                    op1=mybir.AluOpType.add)
            nc.vector.tensor_scalar(o1[:], o1[:], 0.0, 15.0,
                                    op0=mybir.AluOpType.max, op1=mybir.AluOpType.min)
            nc.vector.tensor_scalar(
                ap8[:, n, 1:1 + H, 1:1 + W],
                o1.rearrange("p (h w) -> p h w", w=W), C23, C23,
                op0=mybir.AluOpType.add, op1=mybir.AluOpType.subtract)
            o2 = pools["big"].tile([C, HW], F32, name=f"o2_{n}", tag="bigbuf")
            apn = a1_8[:, n * PW:(n + 1) * PW]
            _conv_image(nc, pools, [apn], w2T, stats2, o2, n, "c2")
            out2.append(o2)

        if DEBUG:
            a1f = pools["stats"].tile([C, PW], F32, name="a1f")
            nc.scalar.copy(a1f[:], a1_8[:, 0:PW])
            nc.sync.dma_start(dbg_a1[:], a1f[:])
            nc.sync.dma_start(dbg_o2[:], out2[0][:])

        # ---- BN2 all-reduce + tail ----
        mean2, rstd2 = _bn_allreduce(nc, pools, stats2, 225.0, 2, epst,
                                     dbg=dbg_st[:, 4:8] if DEBUG else None)
        sc2, bi2 = _affine_vecs(nc, pools, g2, b2, mean2, rstd2, 1.0, 225.0, 2)

        if DEBUG:
            nc.sync.dma_start(dbg_ab[:, 0:1], sc1[:])
            nc.sync.dma_start(dbg_ab[:, 1:2], bi1[:])
            nc.sync.dma_start(dbg_ab[:, 2:3], sc2[:])
            nc.sync.dma_start(dbg_ab[:, 3:4], bi2[:])

        HWH = HW // 2
        for n in range(BPC):
            o2 = out2[n]
            # residual x in two halves (reuses the bf16 xpart slots)
            xra = pools["xpart"].tile([C, HWH], F32, name="xra", tag="xpart")
            xrb = pools["xpart"].tile([C, HWH], F32, name="xrb", tag="xpart")
            xf = x_in[n].rearrange("c h w -> c (h w)")
            nc.scalar.dma_start(xra[:], xf[:, 0:HWH])
            nc.scalar.dma_start(xrb[:], xf[:, HWH:HW])
            # in-place tail on o2: v=o2*sc2 (ACT) ; +bi2+x ; clamp ; rint*15 ;
            # /15 — all exact ALU ops
            nc.scalar.activation(o2[:], o2[:], mybir.ActivationFunctionType.Copy,
                                 scale=sc2[:])
            nc.vector.scalar_tensor_tensor(o2[:, 0:HWH], o2[:, 0:HWH], bi2[:],
                                           xra[:], op0=mybir.AluOpType.add,
                                           op1=mybir.AluOpType.add)
            nc.vector.scalar_tensor_tensor(o2[:, HWH:HW], o2[:, HWH:HW], bi2[:],
                                           xrb[:], op0=mybir.AluOpType.add,
                                           op1=mybir.AluOpType.add)
            nc.vector.tensor_scalar(o2[:], o2[:], 0.0, 1.0,
                                    op0=mybir.AluOpType.max, op1=mybir.AluOpType.min)
            nc.scalar.activation(o2[:], o2[:], mybir.ActivationFunctionType.Copy,
                                 scale=15.0, bias=C23)
            nc.vector.tensor_scalar(o2[:], o2[:], C23, 1.0 / 15.0,
                                    op0=mybir.AluOpType.subtract,
                                    op1=mybir.AluOpType.mult)
            nc.sync.dma_start(out_d[n], o2.rearrange("p (h w) -> p h w", w=W))

    nc.compile()
    return nc


def _get_nc():
    if "nc" not in _CACHE:
        _CACHE["nc"] = build()
    return _CACHE["nc"]


def kernel(x, w1, w2, gamma1, beta1, gamma2, beta2, _trace=False):
    nc = _get_nc()
    x = np.ascontiguousarray(np.asarray(x, dtype=np.float32))
    in_common = {
        "w1": np.ascontiguousarray(np.asarray(w1, np.float32).reshape(C, C * 9)),
        "w2": np.ascontiguousarray(np.asarray(w2, np.float32).reshape(C, C * 9)),
        "gamma1": np.asarray(gamma1, np.float32).reshape(C, 1),
        "beta1": np.asarray(beta1, np.float32).reshape(C, 1),
        "gamma2": np.asarray(gamma2, np.float32).reshape(C, 1),
        "beta2": np.asarray(beta2, np.float32).reshape(C, 1),
    }
    in_maps = [dict(in_common, x=x[c * BPC:(c + 1) * BPC]) for c in range(N_CORES)]
    res = bass_utils.run_bass_kernel_spmd(nc, in_maps, core_ids=list(range(N_CORES)),
                                          trace=_trace)
    out = np.concatenate([res.results[c]["out"] for c in range(N_CORES)], axis=0)
    if _trace:
        _CACHE["last_exec_time_ns"] = res.exec_time_ns
        _CACHE["last_results"] = res
    return out


if __name__ == "__main__":
    nc = build()
    print("built ok")
